# revision 33
# baseline (speedup 1.0000x reference)
"""Trainium2 Bass kernel v4: decoder layer, head-parallel SPMD over 8 cores.

Core c = (batch b = c//2, head-group hg = c%2); 4 heads per core over all
2048 rows, pairwise collectives to average heads, per-core FFN on its own
1024 rows.

v4 over v3:
  - K-projection eliminated: scores = z @ (Wq Wk^T) @ z_k^T with the [E,E]
    product folded on host (D == E), so the key operand of the score matmul
    is the (transposed) layernorm output / raw-k tile directly.
  - fp8 (e4m3) operands for ALL attention matmuls with DoubleRow perf mode
    (two 128-deep K tiles per instruction); FFN stays bf16 for accuracy.
  - AllReduce/ReduceScatter fire per-half as soon as the last head finishes
    the corresponding row blocks (overlapped with remaining attention).
  - FFN weights prefetched during cross-attention; FFN pipelined per
    512-row chunk behind the two ReduceScatter chunks.
"""
import sys
import types

sys.path.insert(0, "/opt/trn_rl_repo")

import numpy as np
import ml_dtypes

import concourse.bacc as bacc
import concourse.tile as tile
from concourse import mybir
from concourse.bass_utils import run_bass_kernel_spmd

BF16 = ml_dtypes.bfloat16
F8 = ml_dtypes.float8_e4m3  # TRN variant: max normal 240
B, L, E, H, D, FW = 4, 2048, 512, 8, 512, 4
HG = H // 2          # 4 heads per core
HID = FW * E
HALF = L // 2
P = 128
EC = E // P          # 4 contraction chunks of 128
KT = L // P          # 16 key tiles
HC = HID // P
LS = L // P          # 16 row slices (full L)
RS8 = HALF // P      # 8 own-row slices (FFN)
NB = 512
SCALE = float(D) ** -0.5
SA = 64.0            # fp8 scale for the folded [E,E] score matrices
SV = 16.0            # fp8 scale for the V projection weights
GROUPS = [[0, 1], [2, 3], [4, 5], [6, 7]]

# packed per-partition bias columns (per-core, 4 heads each)
CQ_S, CQ_C, C1 = 0, 16, 32
NBIAS = 32 + HC  # 48

TRACE = False
_CACHE = {}


def _build():
    f32, bf16 = mybir.dt.float32, mybir.dt.bfloat16
    f8 = mybir.dt.float8e4
    DR = mybir.MatmulPerfMode.DoubleRow
    nc = bacc.Bacc(None, target_bir_lowering=False, debug=False)

    q_in = nc.dram_tensor("q_nat", [L, E], f32, kind="ExternalInput")
    qcv_in = nc.dram_tensor("qcv", [L, E], f32, kind="ExternalInput")
    kT_in = nc.dram_tensor("kT", [E, L], f8, kind="ExternalInput")
    vT_in = nc.dram_tensor("vT", [E, L], f8, kind="ExternalInput")
    tri_in = nc.dram_tensor("trimask", [P, P], f8, kind="ExternalInput")
    a_s_in = nc.dram_tensor("a_s", [HG, E, E], f8, kind="ExternalInput")
    wv_s_in = nc.dram_tensor("wv_s", [HG, E, D], f8, kind="ExternalInput")
    a_c_in = nc.dram_tensor("a_c", [HG, E, E], f8, kind="ExternalInput")
    wv_c_in = nc.dram_tensor("wv_c", [HG, E, D], f8, kind="ExternalInput")
    w1_in = nc.dram_tensor("w1", [E, HID], bf16, kind="ExternalInput")
    w2_in = nc.dram_tensor("w2", [HID, E], bf16, kind="ExternalInput")
    bias_in = nc.dram_tensor("biases", [P, NBIAS], f32, kind="ExternalInput")
    b2_in = nc.dram_tensor("b2rep", [P, E], f32, kind="ExternalInput")
    id_in = nc.dram_tensor("identity", [P, P], bf16, kind="ExternalInput")
    out_d = nc.dram_tensor("out", [HALF, E], f32, kind="ExternalOutput")

    with tile.TileContext(nc) as tc:
        with (
            tc.tile_pool(name="dramb", bufs=1, space="DRAM") as dramb,
            tc.tile_pool(name="consts", bufs=1) as consts,
            tc.tile_pool(name="ps_big", bufs=4, space="PSUM") as psb,
            tc.tile_pool(name="ps_sum", bufs=2, space="PSUM") as pss,
            tc.tile_pool(name="ps_tr", bufs=2, space="PSUM") as pstr,
            tc.tile_pool(name="stats", bufs=6) as statp,
            tc.tile_pool(name="tmps", bufs=4) as tmpp,
        ):
            idt = consts.tile([P, P], bf16)
            nc.sync.dma_start(idt, id_in[:, :])
            tri = consts.tile([P, P], f8)
            nc.sync.dma_start(tri, tri_in[:, :])
            bia = consts.tile([P, NBIAS], f32)
            nc.sync.dma_start(bia, bias_in[:, :])
            b2t = consts.tile([P, E], f32)
            nc.sync.dma_start(b2t, b2_in[:, :])
            eps = consts.tile([P, 1], f32)
            nc.vector.memset(eps, 1e-5)
            eights = consts.tile([P, 1], f8)
            nc.vector.memset(eights, 8.0)

            # DRAM bounce buffers for the collectives
            ar_in = dramb.tile([L, E], bf16)
            ar_out = dramb.tile([L, E], bf16)
            rs_in = dramb.tile([L, E], bf16)
            rs_out = dramb.tile([HALF, E], bf16)

            def ln_normalize(src_ap, out_dt):
                st = statp.tile([P, 6], f32, tag="bnst")
                nc.vector.bn_stats(st, src_ap)
                mv = statp.tile([P, 2], f32, tag="bnmv")
                nc.vector.bn_aggr(mv, st)
                sd = statp.tile([P, 1], f32, tag="bnsd")
                nc.scalar.activation(
                    sd, mv[:, 1:2], mybir.ActivationFunctionType.Sqrt, bias=eps
                )
                rstd = statp.tile([P, 1], f32, tag="bnrs")
                nc.vector.reciprocal(rstd, sd)
                xn = tmpp.tile([P, E], out_dt, tag="xnb")
                nc.vector.tensor_scalar(
                    out=xn,
                    in0=src_ap,
                    scalar1=mv[:, 0:1],
                    scalar2=rstd,
                    op0=mybir.AluOpType.subtract,
                    op1=mybir.AluOpType.mult,
                )
                return xn

            def transpose_into(dst, xn, col):
                # bf16 transpose (fp8 PE-transpose needs stride-2 output);
                # the evac converts to dst's dtype
                for dt in range(EC):
                    pt = pstr.tile([P, P], bf16, tag="ptr")
                    nc.tensor.transpose(pt, xn[:, dt * P : (dt + 1) * P], idt)
                    nc.scalar.activation(
                        dst[:, dt, col : col + P],
                        pt,
                        mybir.ActivationFunctionType.Copy,
                    )

            def v_proj_tile(v_t, xt_v, wv, vt):
                ps = psb.tile([P, NB], f32, tag="psb")
                for ch in range(0, EC, 2):
                    nc.tensor.matmul(
                        ps,
                        xt_v[:, ch : ch + 2, vt * P : (vt + 1) * P],
                        wv[:, ch : ch + 2, :],
                        start=(ch == 0),
                        stop=(ch == EC - 2),
                        perf_mode=DR,
                    )
                nc.scalar.activation(
                    v_t[:, vt, :],
                    ps,
                    mybir.ActivationFunctionType.Identity,
                    scale=1.0 / SV,
                )

            def compute_v(xt_v, wv_d, h):
                wv = wpool.tile([P, EC, D], f8, tag="wv")
                nc.sync.dma_start(wv, wv_d[h].rearrange("(c p) d -> p c d", p=P))
                v_t = kvq.tile([P, KT, D], f8, tag="vtile")
                for vt in range(KT):
                    v_proj_tile(v_t, xt_v, wv, vt)
                return v_t

            def q2_proj_rb(q_t, wa, xt_q, rb, qb_col, h):
                for dt in range(EC):
                    ps = psb.tile([P, NB], f32, tag="psb")
                    for ch in range(0, EC, 2):
                        nc.tensor.matmul(
                            ps,
                            wa[:, ch : ch + 2, dt * P : (dt + 1) * P],
                            xt_q[:, ch : ch + 2, rb * NB : (rb + 1) * NB],
                            start=(ch == 0),
                            stop=(ch == EC - 2),
                            perf_mode=DR,
                        )
                    nc.scalar.activation(
                        q_t[:, dt, rb * NB : (rb + 1) * NB],
                        ps,
                        mybir.ActivationFunctionType.Identity,
                        bias=bia[:, qb_col + h * 4 + dt : qb_col + h * 4 + dt + 1],
                        scale=1.0 / SA,
                    )

            def score_av_rb(q_t, v_t, xt_k, rb, causal):
                """scores -> exp -> rowsum -> AV for one head, one 512-row rb"""
                if causal:
                    score_kts = list(range(min(KT, (rb + 1) * (NB // P))))
                else:
                    score_kts = list(range(KT))
                p_t = ppool.tile([P, KT, NB], f8, tag="ptile")
                for kt in score_kts:
                    ps = psb.tile([P, NB], f32, tag="psb")
                    for ch in range(0, EC, 2):
                        nc.tensor.matmul(
                            ps,
                            xt_k[:, ch : ch + 2, kt * P : (kt + 1) * P],
                            q_t[:, ch : ch + 2, rb * NB : (rb + 1) * NB],
                            start=(ch == 0),
                            stop=(ch == EC - 2),
                            perf_mode=DR,
                        )
                    nc.scalar.activation(
                        p_t[:, kt, :],
                        ps,
                        mybir.ActivationFunctionType.Exp,
                        scale=SCALE,
                    )
                    if causal and kt >= rb * (NB // P):
                        # diagonal 128x128 sub-block: zero keys > row
                        j = kt - rb * (NB // P)
                        nc.vector.tensor_mul(
                            p_t[:, kt, j * P : (j + 1) * P],
                            p_t[:, kt, j * P : (j + 1) * P],
                            tri,
                        )
                for i in range(NB // P):
                    rs = rb * (NB // P) + i
                    n_kt = (rs + 1) if causal else KT
                    pr = pss.tile([P, 1], f32, tag="pssum")
                    for j in range(n_kt):
                        nc.tensor.matmul(
                            pr,
                            p_t[:, j, i * P : (i + 1) * P],
                            eights,
                            start=(j == 0),
                            stop=(j == n_kt - 1),
                        )
                    r8 = statp.tile([P, 1], f32, tag="r8")
                    nc.vector.reciprocal(r8, pr)
                    po = psb.tile([P, NB], f32, tag="psb")
                    npair = n_kt // 2
                    for j in range(npair):
                        nc.tensor.matmul(
                            po,
                            p_t[:, 2 * j : 2 * j + 2, i * P : (i + 1) * P],
                            v_t[:, 2 * j : 2 * j + 2, :],
                            start=(j == 0),
                            stop=(j == npair - 1 and n_kt % 2 == 0),
                            perf_mode=DR,
                        )
                    if n_kt % 2:
                        nc.tensor.matmul(
                            po,
                            p_t[:, n_kt - 1, i * P : (i + 1) * P],
                            v_t[:, n_kt - 1, :],
                            start=(npair == 0),
                            stop=True,
                        )
                    ot = tmpp.tile([P, E], f32, tag="f32s")
                    nc.vector.tensor_scalar_mul(ot, po, r8)
                    nc.vector.tensor_add(acc[:, rs, :], acc[:, rs, :], ot)

            def attention(xt_q, xt_k, xt_v, a_d, wv_d, qb_col, causal,
                          first_head=0, rb_order=None, post_rb=None):
                """heads first_head..3; accumulate softmax@V/8 into acc
                (scores via the host-folded [E,E] per-head matrix)."""
                n_rb = L // NB
                for h in range(first_head, HG):
                    v_t = compute_v(xt_v, wv_d, h)
                    wa = wpool.tile([P, EC, E], f8, tag="wq")
                    nc.sync.dma_start(
                        wa, a_d[h].rearrange("(c p) d -> p c d", p=P)
                    )
                    q_t = kvq.tile([P, EC, L], f8, tag="qtile")
                    for rb in range(n_rb):
                        q2_proj_rb(q_t, wa, xt_q, rb, qb_col, h)

                    for rb in (rb_order if rb_order is not None else range(n_rb)):
                        score_av_rb(q_t, v_t, xt_k, rb, causal)
                        if post_rb is not None:
                            post_rb(h, rb)

            def emit_ar(qtr):
                # quarter = 4 row slices = one rb of 512 rows
                for rs in range(qtr * 4, qtr * 4 + 4):
                    art = tmpp.tile([P, E], bf16, tag="bfs")
                    nc.vector.tensor_copy(art, acc[:, rs, :])
                    nc.sync.dma_start(ar_in[rs * P : (rs + 1) * P, :], art)
                nc.gpsimd.collective_compute(
                    "AllReduce",
                    mybir.AluOpType.add,
                    replica_groups=GROUPS,
                    ins=[ar_in[qtr * NB : (qtr + 1) * NB, :]],
                    outs=[ar_out[qtr * NB : (qtr + 1) * NB, :]],
                )

            def emit_rs(chunk):
                # chunk c: own slices {2c, 2c+1} + partner slices {8+2c, 8+2c+1};
                # first half of the 512-row input goes to the pair's first core
                slices = [2 * chunk, 2 * chunk + 1, 8 + 2 * chunk, 9 + 2 * chunk]
                for idx, rs in enumerate(slices):
                    pos = chunk * 512 + (idx // 2) * 256 + (idx % 2) * P
                    rcast = tmpp.tile([P, E], bf16, tag="bfs")
                    nc.vector.tensor_copy(rcast, acc[:, rs, :])
                    nc.sync.dma_start(rs_in[pos : pos + P, :], rcast)
                nc.gpsimd.collective_compute(
                    "ReduceScatter",
                    mybir.AluOpType.add,
                    replica_groups=GROUPS,
                    ins=[rs_in[chunk * 512 : (chunk + 1) * 512, :]],
                    outs=[rs_out[chunk * 256 : (chunk + 1) * 256, :]],
                )

            # ---------------- phases A-C: attention pools live here ----------------
            attn_pools = (
                tc.tile_pool(name="ffw", bufs=1),
                tc.tile_pool(name="accp", bufs=1),
                tc.tile_pool(name="wpool", bufs=2),
                tc.tile_pool(name="kvq", bufs=2),
                tc.tile_pool(name="ppool", bufs=2),
            )
            ffwp, accp, wpool, kvq, ppool = (p.__enter__() for p in attn_pools)
            ffwp, accp, wpool, kvq, ppool = [
                p for p in (ffwp, accp, wpool, kvq, ppool)
            ]
            # attention partial accumulator over ALL rows, [128, 16, 512] f32
            acc = accp.tile([P, LS, E], f32)
            nc.vector.memset(acc, 0.0)

            # prefetch cross-attention K/V and FFN weights early; the DMAs
            # drain during phase A/B
            with tc.tile_pool(name="xt2", bufs=1) as xt2p, tc.tile_pool(
                name="kvin", bufs=1
            ) as kvinp:
                ktt = kvinp.tile([P, EC, L], f8, tag="ktin")
                nc.sync.dma_start(ktt, kT_in.rearrange("(c p) t -> p c t", p=P))
                vtt = kvinp.tile([P, EC, L], f8, tag="vtin")
                nc.sync.dma_start(vtt, vT_in.rearrange("(c p) t -> p c t", p=P))
                w1t = ffwp.tile([P, EC, HID], bf16, tag="w1t")
                nc.sync.dma_start(w1t, w1_in.rearrange("(c p) d -> p c d", p=P))
                w2t = ffwp.tile([P, HC, E], bf16, tag="w2t")
                nc.sync.dma_start(w2t, w2_in.rearrange("(c p) d -> p c d", p=P))

                # ---------------- phase A: LN1 -> X^T over all rows ------------
                # head-0 V/Q2 projections interleave per slice to fill the PE
                with tc.tile_pool(name="xt1", bufs=1) as xt1p, tc.tile_pool(
                    name="qstream", bufs=3
                ) as qsp:
                    xt = xt1p.tile([P, EC, L], f8)
                    wv0 = wpool.tile([P, EC, D], f8, tag="wv")
                    nc.sync.dma_start(
                        wv0, wv_s_in[0].rearrange("(c p) d -> p c d", p=P)
                    )
                    wa0 = wpool.tile([P, EC, E], f8, tag="wq")
                    nc.sync.dma_start(
                        wa0, a_s_in[0].rearrange("(c p) d -> p c d", p=P)
                    )
                    v0 = kvq.tile([P, KT, D], f8, tag="vtile")
                    q0 = kvq.tile([P, EC, L], f8, tag="qtile")
                    for t in range(LS):
                        qt = qsp.tile([P, E], f32, tag="qs")
                        nc.sync.dma_start(qt, q_in[t * P : (t + 1) * P, :])
                        xn = ln_normalize(qt, bf16)
                        transpose_into(xt, xn, t * P)
                        v_proj_tile(v0, xt, wv0, t)
                        if t % 4 == 3:
                            # head 0 runs fully interleaved with the LN stream:
                            # rb needs only slices <= t, all just produced
                            rb = t // 4
                            q2_proj_rb(q0, wa0, xt, rb, CQ_S, 0)
                            score_av_rb(q0, v0, xt, rb, True)

                    # ------------- phase B: causal self-attention (heads 1-3) --
                    def post_self(h, rb):
                        # AR quarter rb as soon as the last head finishes it
                        if h == HG - 1:
                            emit_ar(rb)

                    attention(xt, xt, xt, a_s_in, wv_s_in, CQ_S, True,
                              first_head=1, post_rb=post_self)

                # ---------------- phase C: x1, LN2, cross-attention ------------
                x2t = xt2p.tile([P, EC, L], f8)
                v0c = compute_v(vtt, wv_c_in, 0)
                wac0 = wpool.tile([P, EC, E], f8, tag="wq")
                nc.sync.dma_start(wac0, a_c_in[0].rearrange("(c p) d -> p c d", p=P))
                q0c = kvq.tile([P, EC, L], f8, tag="qtile")
                # x1 is streamed one row-slice at a time; nothing reads it later
                # (acc gets x1/2, LN2 consumes it, x2 arrives via ReduceScatter).
                # head 0's cross attention interleaves per rb to fill the
                # AR-quarter-gated stalls.
                for rs in range(LS):
                    art = tmpp.tile([P, E], bf16, tag="bfs")
                    nc.sync.dma_start(art, ar_out[rs * P : (rs + 1) * P, :])
                    qcvt = tmpp.tile([P, E], f32, tag="f32s")
                    nc.sync.dma_start(qcvt, qcv_in[rs * P : (rs + 1) * P, :])
                    x1s = tmpp.tile([P, E], f32, tag="f32s")
                    nc.vector.tensor_add(x1s, qcvt, art)
                    xn = ln_normalize(x1s, bf16)
                    transpose_into(x2t, xn, rs * P)
                    # acc <- x1/2 so the pair's ReduceScatter sums to x2
                    nc.vector.tensor_scalar_mul(acc[:, rs, :], x1s, 0.5)
                    if rs % 4 == 3:
                        rb = rs // 4
                        q2_proj_rb(q0c, wac0, x2t, rb, CQ_C, 0)
                        score_av_rb(q0c, v0c, ktt, rb, False)

                def post_cross(h, rb):
                    # chunks 0,1 = row slices {0-3, 8-11} = rb 0,2; with order
                    # [0,2,1,3] they complete after the last head's 2nd rb
                    if h == HG - 1 and rb == 2:
                        emit_rs(0)
                        emit_rs(1)

                attention(x2t, ktt, vtt, a_c_in, wv_c_in, CQ_C, False,
                          first_head=1, rb_order=[0, 2, 1, 3],
                          post_rb=post_cross)
            emit_rs(2)
            emit_rs(3)
            for p in reversed(attn_pools[1:]):
                p.__exit__(None, None, None)

            # ---------------- phase D: FFN on own half, per 512-row chunk ------
            with tc.tile_pool(name="x2p", bufs=1) as x2p, tc.tile_pool(
                name="xt3", bufs=1
            ) as xt3p, tc.tile_pool(name="h1p", bufs=1) as h1p:
                x2b = x2p.tile([P, RS8, E], bf16)
                x2 = x2p.tile([P, RS8, E], f32)
                x3t = xt3p.tile([P, EC, HALF], bf16)
                h1t = h1p.tile([P, HC, HALF], bf16)
                HB = 256  # 2 row slices per pipelined chunk
                for chunk in range(4):
                    for rs in (2 * chunk, 2 * chunk + 1):
                        nc.sync.dma_start(
                            x2b[:, rs, :], rs_out[rs * P : (rs + 1) * P, :]
                        )
                        xn = ln_normalize(x2b[:, rs, :], bf16)
                        transpose_into(x3t, xn, rs * P)
                        nc.vector.tensor_add(x2[:, rs, :], x2b[:, rs, :], b2t)

                    for ht in range(HC):
                        ps = psb.tile([P, NB], f32, tag="psb")
                        for ch in range(EC):
                            nc.tensor.matmul(
                                ps[:, 0:HB],
                                w1t[:, ch, ht * P : (ht + 1) * P],
                                x3t[:, ch, chunk * HB : (chunk + 1) * HB],
                                start=(ch == 0),
                                stop=(ch == EC - 1),
                            )
                        nc.scalar.activation(
                            h1t[:, ht, chunk * HB : (chunk + 1) * HB],
                            ps[:, 0:HB],
                            mybir.ActivationFunctionType.Relu,
                            bias=bia[:, C1 + ht : C1 + ht + 1],
                        )

                    for rs in (2 * chunk, 2 * chunk + 1):
                        ps = psb.tile([P, NB], f32, tag="psb")
                        for ch in range(HC):
                            nc.tensor.matmul(
                                ps,
                                h1t[:, ch, rs * P : (rs + 1) * P],
                                w2t[:, ch, :],
                                start=(ch == 0),
                                stop=(ch == HC - 1),
                            )
                        ot = tmpp.tile([P, E], f32, tag="f32s")
                        nc.vector.tensor_add(ot, ps, x2[:, rs, :])
                        nc.sync.dma_start(out_d[rs * P : (rs + 1) * P, :], ot)
            attn_pools[0].__exit__(None, None, None)

    nc.compile()
    return nc


def _ensure_ntff_hook():
    try:
        from antenv.axon_hooks import get_axon_ntff_profile_hook  # noqa: F401
        return
    except ImportError:
        pass
    import antenv

    mod = types.ModuleType("antenv.axon_hooks")
    _hook = [None]
    mod.set_axon_ntff_profile_hook = lambda h: _hook.__setitem__(0, h)
    mod.get_axon_ntff_profile_hook = lambda: _hook[0]
    sys.modules["antenv.axon_hooks"] = mod
    antenv.axon_hooks = mod
    from trn_agent_boot.trn_boot import _ntff_profile_via_ctypes

    mod.set_axon_ntff_profile_hook(
        _ntff_profile_via_ctypes("/opt/axon/libaxon_pjrt.so")
    )


def _to8(x, scale=1.0):
    return np.clip(np.asarray(x, np.float32) * scale, -240.0, 240.0).astype(F8)


def kernel(**inputs):
    f = np.float32
    q = np.asarray(inputs["q"], f)
    k = np.asarray(inputs["k"], f)
    v = np.asarray(inputs["v"], f)
    Wq_s = np.asarray(inputs["Wq_s"], f)
    Wk_s = np.asarray(inputs["Wk_s"], f)
    Wv_s = np.asarray(inputs["Wv_s"], f)
    Wq_c = np.asarray(inputs["Wq_c"], f)
    Wk_c = np.asarray(inputs["Wk_c"], f)
    Wv_c = np.asarray(inputs["Wv_c"], f)
    W1 = np.asarray(inputs["W1"], f)
    b1 = np.asarray(inputs["b1"], f)
    W2 = np.asarray(inputs["W2"], f)
    b2 = np.asarray(inputs["b2"], f)
    g1 = np.asarray(inputs["g1"], f)
    be1 = np.asarray(inputs["be1"], f)
    g2 = np.asarray(inputs["g2"], f)
    be2 = np.asarray(inputs["be2"], f)
    g3 = np.asarray(inputs["g3"], f)
    be3 = np.asarray(inputs["be3"], f)

    # folded [E,E] score matrices: S = z A z_k^T (+ w . z_key per key)
    A_s = g1[None, :, None] * (Wq_s @ np.swapaxes(Wk_s, 1, 2)) * g1[None, None, :]
    w_s = np.einsum("e,hef->hf", be1, (Wq_s @ np.swapaxes(Wk_s, 1, 2))) * g1[None, :]
    A_c = g2[None, :, None] * (Wq_c @ np.swapaxes(Wk_c, 1, 2))
    w_c = np.einsum("e,hef->hf", be2, (Wq_c @ np.swapaxes(Wk_c, 1, 2)))
    WvsF = Wv_s * g1[None, :, None]
    # V-projection biases contribute mean_h(be1 @ Wv_s[h]) to every attention
    # output row (softmax rows sum to 1); pre-added to q on the host.
    cvbar = np.einsum("e,hed->d", be1, Wv_s) / H

    A_s8 = np.ascontiguousarray(_to8(A_s, SA))
    A_c8 = np.ascontiguousarray(_to8(A_c, SA))
    Wv_s8 = np.ascontiguousarray(_to8(WvsF, SV))
    Wv_c8 = np.ascontiguousarray(_to8(Wv_c, SV))
    W1F = np.ascontiguousarray((W1 * g3[:, None]).astype(BF16))
    c1 = be3 @ W1 + b1
    W2F = np.ascontiguousarray(W2.astype(BF16))

    b2rep = np.broadcast_to(b2[None, :], (P, E)).astype(f).copy()
    ident = np.eye(P, dtype=BF16)
    # tri[key_i, row_j] = 1 where key <= row within a diagonal block
    tri = np.triu(np.ones((P, P), np.float32)).astype(F8)

    in_maps = []
    for core in range(8):
        b, hg = core // 2, core % 2
        hsl = slice(hg * HG, (hg + 1) * HG)
        biases = np.zeros((P, NBIAS), f)
        for h in range(HG):
            for c in range(4):
                biases[:, CQ_S + h * 4 + c] = w_s[hsl][h, c * P : (c + 1) * P]
                biases[:, CQ_C + h * 4 + c] = w_c[hsl][h, c * P : (c + 1) * P]
        for c in range(HC):
            biases[:, C1 + c] = c1[c * P : (c + 1) * P]
        in_maps.append(
            dict(
                q_nat=np.ascontiguousarray(q[b]),
                qcv=np.ascontiguousarray(q[b] + cvbar[None, :]),
                kT=np.ascontiguousarray(_to8(k[b].T)),
                vT=np.ascontiguousarray(_to8(v[b].T)),
                trimask=tri,
                a_s=np.ascontiguousarray(A_s8[hsl]),
                wv_s=np.ascontiguousarray(Wv_s8[hsl]),
                a_c=np.ascontiguousarray(A_c8[hsl]),
                wv_c=np.ascontiguousarray(Wv_c8[hsl]),
                w1=W1F,
                w2=W2F,
                biases=biases,
                b2rep=b2rep,
                identity=ident,
            )
        )

    if "nc" not in _CACHE:
        _CACHE["nc"] = _build()
    nc = _CACHE["nc"]

    kwargs = {}
    if TRACE:
        _ensure_ntff_hook()
        import os as _os

        _os.environ["BASS_PERFETTO_PROFILE_ALL_CORES"] = "1"
        import tempfile

        kwargs = dict(trace=True, tmpdir=tempfile.mkdtemp())
    res = run_bass_kernel_spmd(nc, in_maps, core_ids=list(range(8)), **kwargs)
    _CACHE["last_res"] = res

    out = np.empty((B, L, E), f)
    for core in range(8):
        b, half = core // 2, core % 2
        out[b, half * HALF : (half + 1) * HALF] = res.results[core]["out"]
    return out


# revision 34
# speedup vs baseline: 1.1971x; 1.1971x over previous
"""Trainium2 Bass kernel v4: decoder layer, head-parallel SPMD over 8 cores.

Core c = (batch b = c//2, head-group hg = c%2); 4 heads per core over all
2048 rows, pairwise collectives to average heads, per-core FFN on its own
1024 rows.

v4 over v3:
  - K-projection eliminated: scores = z @ (Wq Wk^T) @ z_k^T with the [E,E]
    product folded on host (D == E), so the key operand of the score matmul
    is the (transposed) layernorm output / raw-k tile directly.
  - fp8 (e4m3) operands for ALL attention matmuls with DoubleRow perf mode
    (two 128-deep K tiles per instruction); FFN stays bf16 for accuracy.
  - AllReduce/ReduceScatter fire per-half as soon as the last head finishes
    the corresponding row blocks (overlapped with remaining attention).
  - FFN weights prefetched during cross-attention; FFN pipelined per
    512-row chunk behind the two ReduceScatter chunks.
"""
import sys
import types

sys.path.insert(0, "/opt/trn_rl_repo")

import numpy as np
import ml_dtypes

import concourse.bacc as bacc
import concourse.tile as tile
from concourse import mybir
from concourse.bass_utils import run_bass_kernel_spmd

BF16 = ml_dtypes.bfloat16
F8 = ml_dtypes.float8_e4m3  # TRN variant: max normal 240
B, L, E, H, D, FW = 4, 2048, 512, 8, 512, 4
HG = H // 2          # 4 heads per core
HID = FW * E
HALF = L // 2
P = 128
EC = E // P          # 4 contraction chunks of 128
KT = L // P          # 16 key tiles
HC = HID // P
LS = L // P          # 16 row slices (full L)
RS8 = HALF // P      # 8 own-row slices (FFN)
NB = 512
SCALE = float(D) ** -0.5
SA = 64.0            # fp8 scale for the folded [E,E] score matrices
SV = 16.0            # fp8 scale for the V projection weights
GROUPS = [[0, 1], [2, 3], [4, 5], [6, 7]]

# packed per-partition bias columns (per-core, 4 heads each)
CQ_S, CQ_C, C1 = 0, 16, 32
NBIAS = 32 + HC  # 48

TRACE = False
_CACHE = {}


def _build():
    f32, bf16 = mybir.dt.float32, mybir.dt.bfloat16
    f8 = mybir.dt.float8e4
    DR = mybir.MatmulPerfMode.DoubleRow
    nc = bacc.Bacc(None, target_bir_lowering=False, debug=False)

    q_in = nc.dram_tensor("q_nat", [L, E], f32, kind="ExternalInput")
    qcv_in = nc.dram_tensor("qcv", [L, E], f32, kind="ExternalInput")
    kT_in = nc.dram_tensor("kT", [E, L], f8, kind="ExternalInput")
    vT_in = nc.dram_tensor("vT", [E, L], f8, kind="ExternalInput")
    tri_in = nc.dram_tensor("trimask", [P, P], f8, kind="ExternalInput")
    a_s_in = nc.dram_tensor("a_s", [HG, E, E], f8, kind="ExternalInput")
    wv_s_in = nc.dram_tensor("wv_s", [HG, E, D], f8, kind="ExternalInput")
    a_c_in = nc.dram_tensor("a_c", [HG, E, E], f8, kind="ExternalInput")
    wv_c_in = nc.dram_tensor("wv_c", [HG, E, D], f8, kind="ExternalInput")
    w1_in = nc.dram_tensor("w1", [E, HID], bf16, kind="ExternalInput")
    w2_in = nc.dram_tensor("w2", [HID, E], bf16, kind="ExternalInput")
    bias_in = nc.dram_tensor("biases", [P, NBIAS], f32, kind="ExternalInput")
    b2_in = nc.dram_tensor("b2rep", [P, E], f32, kind="ExternalInput")
    id_in = nc.dram_tensor("identity", [P, P], bf16, kind="ExternalInput")
    out_d = nc.dram_tensor("out", [HALF, E], f32, kind="ExternalOutput")

    with tile.TileContext(nc) as tc:
        with (
            tc.tile_pool(name="dramb", bufs=1, space="DRAM") as dramb,
            tc.tile_pool(name="consts", bufs=1) as consts,
            tc.tile_pool(name="ps_big", bufs=4, space="PSUM") as psb,
            tc.tile_pool(name="ps_sum", bufs=2, space="PSUM") as pss,
            tc.tile_pool(name="ps_tr", bufs=2, space="PSUM") as pstr,
            tc.tile_pool(name="stats", bufs=6) as statp,
            tc.tile_pool(name="tmps", bufs=4) as tmpp,
        ):
            idt = consts.tile([P, P], bf16)
            nc.sync.dma_start(idt, id_in[:, :])
            tri = consts.tile([P, P], f8)
            nc.sync.dma_start(tri, tri_in[:, :])
            bia = consts.tile([P, NBIAS], f32)
            nc.sync.dma_start(bia, bias_in[:, :])
            b2t = consts.tile([P, E], f32)
            nc.sync.dma_start(b2t, b2_in[:, :])
            eps = consts.tile([P, 1], f32)
            nc.vector.memset(eps, 1e-5)
            eights = consts.tile([P, 1], f8)
            nc.vector.memset(eights, 8.0)

            # DRAM bounce buffers for the collectives
            ar_in = dramb.tile([L, E], bf16)
            ar_out = dramb.tile([L, E], bf16)
            rs_in = dramb.tile([L, E], bf16)
            rs_out = dramb.tile([HALF, E], bf16)

            def ln_normalize(src_ap, out_dt):
                st = statp.tile([P, 6], f32, tag="bnst")
                nc.vector.bn_stats(st, src_ap)
                mv = statp.tile([P, 2], f32, tag="bnmv")
                nc.vector.bn_aggr(mv, st)
                sd = statp.tile([P, 1], f32, tag="bnsd")
                nc.scalar.activation(
                    sd, mv[:, 1:2], mybir.ActivationFunctionType.Sqrt, bias=eps
                )
                rstd = statp.tile([P, 1], f32, tag="bnrs")
                nc.vector.reciprocal(rstd, sd)
                xn = tmpp.tile([P, E], out_dt, tag="xnb")
                nc.vector.tensor_scalar(
                    out=xn,
                    in0=src_ap,
                    scalar1=mv[:, 0:1],
                    scalar2=rstd,
                    op0=mybir.AluOpType.subtract,
                    op1=mybir.AluOpType.mult,
                )
                return xn

            def transpose_into(dst, xn, col):
                # bf16 transpose (fp8 PE-transpose needs stride-2 output);
                # the evac converts to dst's dtype
                for dt in range(EC):
                    pt = pstr.tile([P, P], bf16, tag="ptr")
                    nc.tensor.transpose(pt, xn[:, dt * P : (dt + 1) * P], idt)
                    nc.scalar.activation(
                        dst[:, dt, col : col + P],
                        pt,
                        mybir.ActivationFunctionType.Copy,
                    )

            def v_proj_tile(v_t, xt_v, wv, vt):
                ps = psb.tile([P, NB], f32, tag="psb")
                for ch in range(0, EC, 2):
                    nc.tensor.matmul(
                        ps,
                        xt_v[:, ch : ch + 2, vt * P : (vt + 1) * P],
                        wv[:, ch : ch + 2, :],
                        start=(ch == 0),
                        stop=(ch == EC - 2),
                        perf_mode=DR,
                    )
                nc.scalar.activation(
                    v_t[:, vt, :],
                    ps,
                    mybir.ActivationFunctionType.Identity,
                    scale=1.0 / SV,
                )

            def compute_v(xt_v, wv_d, h):
                wv = wpool.tile([P, EC, D], f8, tag="wv")
                nc.sync.dma_start(wv, wv_d[h].rearrange("(c p) d -> p c d", p=P))
                v_t = kvq.tile([P, KT, D], f8, tag="vtile")
                for vt in range(KT):
                    v_proj_tile(v_t, xt_v, wv, vt)
                return v_t

            def q2_proj_rb(q_t, wa, xt_q, rb, qb_col, h):
                for dt in range(EC):
                    ps = psb.tile([P, NB], f32, tag="psb")
                    for ch in range(0, EC, 2):
                        nc.tensor.matmul(
                            ps,
                            wa[:, ch : ch + 2, dt * P : (dt + 1) * P],
                            xt_q[:, ch : ch + 2, rb * NB : (rb + 1) * NB],
                            start=(ch == 0),
                            stop=(ch == EC - 2),
                            perf_mode=DR,
                        )
                    nc.scalar.activation(
                        q_t[:, dt, rb * NB : (rb + 1) * NB],
                        ps,
                        mybir.ActivationFunctionType.Identity,
                        bias=bia[:, qb_col + h * 4 + dt : qb_col + h * 4 + dt + 1],
                        scale=1.0 / SA,
                    )

            def attention(xt_q, xt_k, xt_v, a_d, wv_d, qb_col, causal,
                          pre_v=None, pre_q=None, rb_order=None, post_rb=None):
                """4 heads; accumulate softmax@V/8 into acc (scores via the
                host-folded [E,E] per-head matrix; keys operand = xt_k)."""
                n_rb = L // NB
                for h in range(HG):
                    if h == 0 and pre_v is not None:
                        v_t = pre_v
                    else:
                        v_t = compute_v(xt_v, wv_d, h)
                    if h == 0 and pre_q is not None:
                        q_t = pre_q
                    else:
                        wa = wpool.tile([P, EC, E], f8, tag="wq")
                        nc.sync.dma_start(
                            wa, a_d[h].rearrange("(c p) d -> p c d", p=P)
                        )
                        q_t = kvq.tile([P, EC, L], f8, tag="qtile")
                        for rb in range(n_rb):
                            q2_proj_rb(q_t, wa, xt_q, rb, qb_col, h)

                    for rb in (rb_order if rb_order is not None else range(n_rb)):
                        if causal:
                            score_kts = list(range(min(KT, (rb + 1) * (NB // P))))
                        else:
                            score_kts = list(range(KT))
                        p_t = ppool.tile([P, KT, NB], f8, tag="ptile")
                        for kt in score_kts:
                            ps = psb.tile([P, NB], f32, tag="psb")
                            for ch in range(0, EC, 2):
                                nc.tensor.matmul(
                                    ps,
                                    xt_k[:, ch : ch + 2, kt * P : (kt + 1) * P],
                                    q_t[:, ch : ch + 2, rb * NB : (rb + 1) * NB],
                                    start=(ch == 0),
                                    stop=(ch == EC - 2),
                                    perf_mode=DR,
                                )
                            nc.scalar.activation(
                                p_t[:, kt, :],
                                ps,
                                mybir.ActivationFunctionType.Exp,
                                scale=SCALE,
                            )
                            if causal and kt >= rb * (NB // P):
                                # diagonal 128x128 sub-block: zero keys > row
                                j = kt - rb * (NB // P)
                                nc.vector.tensor_mul(
                                    p_t[:, kt, j * P : (j + 1) * P],
                                    p_t[:, kt, j * P : (j + 1) * P],
                                    tri,
                                )
                        for i in range(NB // P):
                            rs = rb * (NB // P) + i
                            n_kt = (rs + 1) if causal else KT
                            pr = pss.tile([P, 1], f32, tag="pssum")
                            for j in range(n_kt):
                                nc.tensor.matmul(
                                    pr,
                                    p_t[:, j, i * P : (i + 1) * P],
                                    eights,
                                    start=(j == 0),
                                    stop=(j == n_kt - 1),
                                )
                            r8 = statp.tile([P, 1], f32, tag="r8")
                            nc.vector.reciprocal(r8, pr)
                            po = psb.tile([P, NB], f32, tag="psb")
                            npair = n_kt // 2
                            for j in range(npair):
                                nc.tensor.matmul(
                                    po,
                                    p_t[:, 2 * j : 2 * j + 2, i * P : (i + 1) * P],
                                    v_t[:, 2 * j : 2 * j + 2, :],
                                    start=(j == 0),
                                    stop=(j == npair - 1 and n_kt % 2 == 0),
                                    perf_mode=DR,
                                )
                            if n_kt % 2:
                                nc.tensor.matmul(
                                    po,
                                    p_t[:, n_kt - 1, i * P : (i + 1) * P],
                                    v_t[:, n_kt - 1, :],
                                    start=(npair == 0),
                                    stop=True,
                                )
                            ot = tmpp.tile([P, E], f32, tag="f32s")
                            nc.vector.tensor_scalar_mul(ot, po, r8)
                            nc.vector.tensor_add(acc[:, rs, :], acc[:, rs, :], ot)
                        if post_rb is not None:
                            post_rb(h, rb)

            def emit_ar(qtr):
                # quarter = 4 row slices = one rb of 512 rows
                for rs in range(qtr * 4, qtr * 4 + 4):
                    art = tmpp.tile([P, E], bf16, tag="bfs")
                    nc.vector.tensor_copy(art, acc[:, rs, :])
                    nc.sync.dma_start(ar_in[rs * P : (rs + 1) * P, :], art)
                nc.gpsimd.collective_compute(
                    "AllReduce",
                    mybir.AluOpType.add,
                    replica_groups=GROUPS,
                    ins=[ar_in[qtr * NB : (qtr + 1) * NB, :]],
                    outs=[ar_out[qtr * NB : (qtr + 1) * NB, :]],
                )

            def emit_rs(chunk):
                # chunk c: own slices {2c, 2c+1} + partner slices {8+2c, 8+2c+1};
                # first half of the 512-row input goes to the pair's first core
                slices = [2 * chunk, 2 * chunk + 1, 8 + 2 * chunk, 9 + 2 * chunk]
                for idx, rs in enumerate(slices):
                    pos = chunk * 512 + (idx // 2) * 256 + (idx % 2) * P
                    rcast = tmpp.tile([P, E], bf16, tag="bfs")
                    nc.vector.tensor_copy(rcast, acc[:, rs, :])
                    nc.sync.dma_start(rs_in[pos : pos + P, :], rcast)
                nc.gpsimd.collective_compute(
                    "ReduceScatter",
                    mybir.AluOpType.add,
                    replica_groups=GROUPS,
                    ins=[rs_in[chunk * 512 : (chunk + 1) * 512, :]],
                    outs=[rs_out[chunk * 256 : (chunk + 1) * 256, :]],
                )

            # ---------------- phases A-C: attention pools live here ----------------
            attn_pools = (
                tc.tile_pool(name="ffw", bufs=1),
                tc.tile_pool(name="accp", bufs=1),
                tc.tile_pool(name="wpool", bufs=2),
                tc.tile_pool(name="kvq", bufs=2),
                tc.tile_pool(name="ppool", bufs=2),
            )
            ffwp, accp, wpool, kvq, ppool = (p.__enter__() for p in attn_pools)
            ffwp, accp, wpool, kvq, ppool = [
                p for p in (ffwp, accp, wpool, kvq, ppool)
            ]
            # attention partial accumulator over ALL rows, [128, 16, 512] f32
            acc = accp.tile([P, LS, E], f32)
            nc.vector.memset(acc, 0.0)

            # prefetch cross-attention K/V and FFN weights early; the DMAs
            # drain during phase A/B
            with tc.tile_pool(name="xt2", bufs=1) as xt2p, tc.tile_pool(
                name="kvin", bufs=1
            ) as kvinp:
                ktt = kvinp.tile([P, EC, L], f8, tag="ktin")
                nc.sync.dma_start(ktt, kT_in.rearrange("(c p) t -> p c t", p=P))
                vtt = kvinp.tile([P, EC, L], f8, tag="vtin")
                nc.sync.dma_start(vtt, vT_in.rearrange("(c p) t -> p c t", p=P))
                w1t = ffwp.tile([P, EC, HID], bf16, tag="w1t")
                nc.sync.dma_start(w1t, w1_in.rearrange("(c p) d -> p c d", p=P))
                w2t = ffwp.tile([P, HC, E], bf16, tag="w2t")
                nc.sync.dma_start(w2t, w2_in.rearrange("(c p) d -> p c d", p=P))

                # ---------------- phase A: LN1 -> X^T over all rows ------------
                # head-0 V/Q2 projections interleave per slice to fill the PE
                with tc.tile_pool(name="xt1", bufs=1) as xt1p, tc.tile_pool(
                    name="qstream", bufs=3
                ) as qsp:
                    xt = xt1p.tile([P, EC, L], f8)
                    wv0 = wpool.tile([P, EC, D], f8, tag="wv")
                    nc.sync.dma_start(
                        wv0, wv_s_in[0].rearrange("(c p) d -> p c d", p=P)
                    )
                    wa0 = wpool.tile([P, EC, E], f8, tag="wq")
                    nc.sync.dma_start(
                        wa0, a_s_in[0].rearrange("(c p) d -> p c d", p=P)
                    )
                    v0 = kvq.tile([P, KT, D], f8, tag="vtile")
                    q0 = kvq.tile([P, EC, L], f8, tag="qtile")
                    for t in range(LS):
                        qt = qsp.tile([P, E], f32, tag="qs")
                        nc.sync.dma_start(qt, q_in[t * P : (t + 1) * P, :])
                        xn = ln_normalize(qt, bf16)
                        transpose_into(xt, xn, t * P)
                        v_proj_tile(v0, xt, wv0, t)
                        if t % 4 == 3:
                            q2_proj_rb(q0, wa0, xt, t // 4, CQ_S, 0)

                    # ------------- phase B: causal self-attention (4 heads) ----
                    def post_self(h, rb):
                        # AR quarter rb as soon as the last head finishes it
                        if h == HG - 1:
                            emit_ar(rb)

                    attention(xt, xt, xt, a_s_in, wv_s_in, CQ_S, True,
                              pre_v=v0, pre_q=q0, post_rb=post_self)

                # ---------------- phase C: x1, LN2, cross-attention ------------
                x2t = xt2p.tile([P, EC, L], f8)
                pre = compute_v(vtt, wv_c_in, 0)
                wac0 = wpool.tile([P, EC, E], f8, tag="wq")
                nc.sync.dma_start(wac0, a_c_in[0].rearrange("(c p) d -> p c d", p=P))
                q0c = kvq.tile([P, EC, L], f8, tag="qtile")
                # x1 is streamed one row-slice at a time; nothing reads it later
                # (acc gets x1/2, LN2 consumes it, x2 arrives via ReduceScatter)
                for rs in range(LS):
                    art = tmpp.tile([P, E], bf16, tag="bfs")
                    nc.sync.dma_start(art, ar_out[rs * P : (rs + 1) * P, :])
                    qcvt = tmpp.tile([P, E], f32, tag="f32s")
                    nc.sync.dma_start(qcvt, qcv_in[rs * P : (rs + 1) * P, :])
                    x1s = tmpp.tile([P, E], f32, tag="f32s")
                    nc.vector.tensor_add(x1s, qcvt, art)
                    xn = ln_normalize(x1s, bf16)
                    transpose_into(x2t, xn, rs * P)
                    # acc <- x1/2 so the pair's ReduceScatter sums to x2
                    nc.vector.tensor_scalar_mul(acc[:, rs, :], x1s, 0.5)
                    if rs % 4 == 3:
                        q2_proj_rb(q0c, wac0, x2t, rs // 4, CQ_C, 0)

                def post_cross(h, rb):
                    # chunks 0,1 = row slices {0-3, 8-11} = rb 0,2; with order
                    # [0,2,1,3] they complete after the last head's 2nd rb
                    if h == HG - 1 and rb == 2:
                        emit_rs(0)
                        emit_rs(1)

                attention(x2t, ktt, vtt, a_c_in, wv_c_in, CQ_C, False,
                          pre_v=pre, pre_q=q0c, rb_order=[0, 2, 1, 3],
                          post_rb=post_cross)
            emit_rs(2)
            emit_rs(3)
            for p in reversed(attn_pools[1:]):
                p.__exit__(None, None, None)

            # ---------------- phase D: FFN on own half, per 512-row chunk ------
            with tc.tile_pool(name="x2p", bufs=1) as x2p, tc.tile_pool(
                name="xt3", bufs=1
            ) as xt3p, tc.tile_pool(name="h1p", bufs=1) as h1p:
                x2b = x2p.tile([P, RS8, E], bf16)
                x2 = x2p.tile([P, RS8, E], f32)
                x3t = xt3p.tile([P, EC, HALF], bf16)
                h1t = h1p.tile([P, HC, HALF], bf16)
                HB = 256  # 2 row slices per pipelined chunk
                for chunk in range(4):
                    for rs in (2 * chunk, 2 * chunk + 1):
                        nc.sync.dma_start(
                            x2b[:, rs, :], rs_out[rs * P : (rs + 1) * P, :]
                        )
                        xn = ln_normalize(x2b[:, rs, :], bf16)
                        transpose_into(x3t, xn, rs * P)
                        nc.vector.tensor_add(x2[:, rs, :], x2b[:, rs, :], b2t)

                    for ht in range(HC):
                        ps = psb.tile([P, NB], f32, tag="psb")
                        for ch in range(EC):
                            nc.tensor.matmul(
                                ps[:, 0:HB],
                                w1t[:, ch, ht * P : (ht + 1) * P],
                                x3t[:, ch, chunk * HB : (chunk + 1) * HB],
                                start=(ch == 0),
                                stop=(ch == EC - 1),
                            )
                        nc.scalar.activation(
                            h1t[:, ht, chunk * HB : (chunk + 1) * HB],
                            ps[:, 0:HB],
                            mybir.ActivationFunctionType.Relu,
                            bias=bia[:, C1 + ht : C1 + ht + 1],
                        )

                    for rs in (2 * chunk, 2 * chunk + 1):
                        ps = psb.tile([P, NB], f32, tag="psb")
                        for ch in range(HC):
                            nc.tensor.matmul(
                                ps,
                                h1t[:, ch, rs * P : (rs + 1) * P],
                                w2t[:, ch, :],
                                start=(ch == 0),
                                stop=(ch == HC - 1),
                            )
                        ot = tmpp.tile([P, E], f32, tag="f32s")
                        nc.vector.tensor_add(ot, ps, x2[:, rs, :])
                        nc.sync.dma_start(out_d[rs * P : (rs + 1) * P, :], ot)
            attn_pools[0].__exit__(None, None, None)

    nc.compile()
    return nc


def _ensure_ntff_hook():
    try:
        from antenv.axon_hooks import get_axon_ntff_profile_hook  # noqa: F401
        return
    except ImportError:
        pass
    import antenv

    mod = types.ModuleType("antenv.axon_hooks")
    _hook = [None]
    mod.set_axon_ntff_profile_hook = lambda h: _hook.__setitem__(0, h)
    mod.get_axon_ntff_profile_hook = lambda: _hook[0]
    sys.modules["antenv.axon_hooks"] = mod
    antenv.axon_hooks = mod
    from trn_agent_boot.trn_boot import _ntff_profile_via_ctypes

    mod.set_axon_ntff_profile_hook(
        _ntff_profile_via_ctypes("/opt/axon/libaxon_pjrt.so")
    )


def _to8(x, scale=1.0):
    return np.clip(np.asarray(x, np.float32) * scale, -240.0, 240.0).astype(F8)


def kernel(**inputs):
    f = np.float32
    q = np.asarray(inputs["q"], f)
    k = np.asarray(inputs["k"], f)
    v = np.asarray(inputs["v"], f)
    Wq_s = np.asarray(inputs["Wq_s"], f)
    Wk_s = np.asarray(inputs["Wk_s"], f)
    Wv_s = np.asarray(inputs["Wv_s"], f)
    Wq_c = np.asarray(inputs["Wq_c"], f)
    Wk_c = np.asarray(inputs["Wk_c"], f)
    Wv_c = np.asarray(inputs["Wv_c"], f)
    W1 = np.asarray(inputs["W1"], f)
    b1 = np.asarray(inputs["b1"], f)
    W2 = np.asarray(inputs["W2"], f)
    b2 = np.asarray(inputs["b2"], f)
    g1 = np.asarray(inputs["g1"], f)
    be1 = np.asarray(inputs["be1"], f)
    g2 = np.asarray(inputs["g2"], f)
    be2 = np.asarray(inputs["be2"], f)
    g3 = np.asarray(inputs["g3"], f)
    be3 = np.asarray(inputs["be3"], f)

    # folded [E,E] score matrices: S = z A z_k^T (+ w . z_key per key)
    A_s = g1[None, :, None] * (Wq_s @ np.swapaxes(Wk_s, 1, 2)) * g1[None, None, :]
    w_s = np.einsum("e,hef->hf", be1, (Wq_s @ np.swapaxes(Wk_s, 1, 2))) * g1[None, :]
    A_c = g2[None, :, None] * (Wq_c @ np.swapaxes(Wk_c, 1, 2))
    w_c = np.einsum("e,hef->hf", be2, (Wq_c @ np.swapaxes(Wk_c, 1, 2)))
    WvsF = Wv_s * g1[None, :, None]
    # V-projection biases contribute mean_h(be1 @ Wv_s[h]) to every attention
    # output row (softmax rows sum to 1); pre-added to q on the host.
    cvbar = np.einsum("e,hed->d", be1, Wv_s) / H

    A_s8 = np.ascontiguousarray(_to8(A_s, SA))
    A_c8 = np.ascontiguousarray(_to8(A_c, SA))
    Wv_s8 = np.ascontiguousarray(_to8(WvsF, SV))
    Wv_c8 = np.ascontiguousarray(_to8(Wv_c, SV))
    W1F = np.ascontiguousarray((W1 * g3[:, None]).astype(BF16))
    c1 = be3 @ W1 + b1
    W2F = np.ascontiguousarray(W2.astype(BF16))

    b2rep = np.broadcast_to(b2[None, :], (P, E)).astype(f).copy()
    ident = np.eye(P, dtype=BF16)
    # tri[key_i, row_j] = 1 where key <= row within a diagonal block
    tri = np.triu(np.ones((P, P), np.float32)).astype(F8)

    in_maps = []
    for core in range(8):
        b, hg = core // 2, core % 2
        hsl = slice(hg * HG, (hg + 1) * HG)
        biases = np.zeros((P, NBIAS), f)
        for h in range(HG):
            for c in range(4):
                biases[:, CQ_S + h * 4 + c] = w_s[hsl][h, c * P : (c + 1) * P]
                biases[:, CQ_C + h * 4 + c] = w_c[hsl][h, c * P : (c + 1) * P]
        for c in range(HC):
            biases[:, C1 + c] = c1[c * P : (c + 1) * P]
        in_maps.append(
            dict(
                q_nat=np.ascontiguousarray(q[b]),
                qcv=np.ascontiguousarray(q[b] + cvbar[None, :]),
                kT=np.ascontiguousarray(_to8(k[b].T)),
                vT=np.ascontiguousarray(_to8(v[b].T)),
                trimask=tri,
                a_s=np.ascontiguousarray(A_s8[hsl]),
                wv_s=np.ascontiguousarray(Wv_s8[hsl]),
                a_c=np.ascontiguousarray(A_c8[hsl]),
                wv_c=np.ascontiguousarray(Wv_c8[hsl]),
                w1=W1F,
                w2=W2F,
                biases=biases,
                b2rep=b2rep,
                identity=ident,
            )
        )

    if "nc" not in _CACHE:
        _CACHE["nc"] = _build()
    nc = _CACHE["nc"]

    kwargs = {}
    if TRACE:
        _ensure_ntff_hook()
        import os as _os

        _os.environ["BASS_PERFETTO_PROFILE_ALL_CORES"] = "1"
        import tempfile

        kwargs = dict(trace=True, tmpdir=tempfile.mkdtemp())
    res = run_bass_kernel_spmd(nc, in_maps, core_ids=list(range(8)), **kwargs)
    _CACHE["last_res"] = res

    out = np.empty((B, L, E), f)
    for core in range(8):
        b, half = core // 2, core % 2
        out[b, half * HALF : (half + 1) * HALF] = res.results[core]["out"]
    return out


# revision 35
# speedup vs baseline: 1.1993x; 1.0019x over previous
"""Trainium2 Bass kernel v4: decoder layer, head-parallel SPMD over 8 cores.

Core c = (batch b = c//2, head-group hg = c%2); 4 heads per core over all
2048 rows, pairwise collectives to average heads, per-core FFN on its own
1024 rows.

v4 over v3:
  - K-projection eliminated: scores = z @ (Wq Wk^T) @ z_k^T with the [E,E]
    product folded on host (D == E), so the key operand of the score matmul
    is the (transposed) layernorm output / raw-k tile directly.
  - fp8 (e4m3) operands for ALL attention matmuls with DoubleRow perf mode
    (two 128-deep K tiles per instruction); FFN stays bf16 for accuracy.
  - AllReduce/ReduceScatter fire per-half as soon as the last head finishes
    the corresponding row blocks (overlapped with remaining attention).
  - FFN weights prefetched during cross-attention; FFN pipelined per
    512-row chunk behind the two ReduceScatter chunks.
"""
import sys
import types

sys.path.insert(0, "/opt/trn_rl_repo")

import numpy as np
import ml_dtypes

import concourse.bacc as bacc
import concourse.tile as tile
from concourse import mybir
from concourse.bass_utils import run_bass_kernel_spmd

BF16 = ml_dtypes.bfloat16
F8 = ml_dtypes.float8_e4m3  # TRN variant: max normal 240
B, L, E, H, D, FW = 4, 2048, 512, 8, 512, 4
HG = H // 2          # 4 heads per core
HID = FW * E
HALF = L // 2
P = 128
EC = E // P          # 4 contraction chunks of 128
KT = L // P          # 16 key tiles
HC = HID // P
LS = L // P          # 16 row slices (full L)
RS8 = HALF // P      # 8 own-row slices (FFN)
NB = 512
SCALE = float(D) ** -0.5
SA = 64.0            # fp8 scale for the folded [E,E] score matrices
SV = 16.0            # fp8 scale for the V projection weights
GROUPS = [[0, 1], [2, 3], [4, 5], [6, 7]]

# packed per-partition bias columns (per-core, 4 heads each)
CQ_S, CQ_C, C1 = 0, 16, 32
NBIAS = 32 + HC  # 48

TRACE = False
_CACHE = {}


def _build():
    f32, bf16 = mybir.dt.float32, mybir.dt.bfloat16
    f8 = mybir.dt.float8e4
    DR = mybir.MatmulPerfMode.DoubleRow
    nc = bacc.Bacc(None, target_bir_lowering=False, debug=False)

    q_in = nc.dram_tensor("q_nat", [L, E], f32, kind="ExternalInput")
    qcv_in = nc.dram_tensor("qcv", [L, E], f32, kind="ExternalInput")
    kT_in = nc.dram_tensor("kT", [E, L], f8, kind="ExternalInput")
    vT_in = nc.dram_tensor("vT", [E, L], f8, kind="ExternalInput")
    tri_in = nc.dram_tensor("trimask", [P, P], f8, kind="ExternalInput")
    a_s_in = nc.dram_tensor("a_s", [HG, E, E], f8, kind="ExternalInput")
    wv_s_in = nc.dram_tensor("wv_s", [HG, E, D], f8, kind="ExternalInput")
    a_c_in = nc.dram_tensor("a_c", [HG, E, E], f8, kind="ExternalInput")
    wv_c_in = nc.dram_tensor("wv_c", [HG, E, D], f8, kind="ExternalInput")
    w1_in = nc.dram_tensor("w1", [E, HID], bf16, kind="ExternalInput")
    w2_in = nc.dram_tensor("w2", [HID, E], bf16, kind="ExternalInput")
    bias_in = nc.dram_tensor("biases", [P, NBIAS], f32, kind="ExternalInput")
    b2_in = nc.dram_tensor("b2rep", [P, E], f32, kind="ExternalInput")
    id_in = nc.dram_tensor("identity", [P, P], bf16, kind="ExternalInput")
    out_d = nc.dram_tensor("out", [HALF, E], f32, kind="ExternalOutput")

    with tile.TileContext(nc) as tc:
        with (
            tc.tile_pool(name="dramb", bufs=1, space="DRAM") as dramb,
            tc.tile_pool(name="consts", bufs=1) as consts,
            tc.tile_pool(name="ps_big", bufs=4, space="PSUM") as psb,
            tc.tile_pool(name="ps_sum", bufs=2, space="PSUM") as pss,
            tc.tile_pool(name="ps_tr", bufs=2, space="PSUM") as pstr,
            tc.tile_pool(name="stats", bufs=6) as statp,
            tc.tile_pool(name="tmps", bufs=4) as tmpp,
        ):
            idt = consts.tile([P, P], bf16)
            nc.sync.dma_start(idt, id_in[:, :])
            tri = consts.tile([P, P], f8)
            nc.sync.dma_start(tri, tri_in[:, :])
            bia = consts.tile([P, NBIAS], f32)
            nc.sync.dma_start(bia, bias_in[:, :])
            b2t = consts.tile([P, E], f32)
            nc.sync.dma_start(b2t, b2_in[:, :])
            eps = consts.tile([P, 1], f32)
            nc.vector.memset(eps, 1e-5)
            eights = consts.tile([P, 1], f8)
            nc.vector.memset(eights, 8.0)

            # DRAM bounce buffers for the collectives
            ar_in = dramb.tile([L, E], bf16)
            ar_out = dramb.tile([L, E], bf16)
            rs_in = dramb.tile([L, E], bf16)
            rs_out = dramb.tile([HALF, E], bf16)

            def ln_normalize(src_ap, out_dt):
                st = statp.tile([P, 6], f32, tag="bnst")
                nc.vector.bn_stats(st, src_ap)
                mv = statp.tile([P, 2], f32, tag="bnmv")
                nc.vector.bn_aggr(mv, st)
                sd = statp.tile([P, 1], f32, tag="bnsd")
                nc.scalar.activation(
                    sd, mv[:, 1:2], mybir.ActivationFunctionType.Sqrt, bias=eps
                )
                rstd = statp.tile([P, 1], f32, tag="bnrs")
                nc.vector.reciprocal(rstd, sd)
                xn = tmpp.tile([P, E], out_dt, tag="xnb")
                nc.vector.tensor_scalar(
                    out=xn,
                    in0=src_ap,
                    scalar1=mv[:, 0:1],
                    scalar2=rstd,
                    op0=mybir.AluOpType.subtract,
                    op1=mybir.AluOpType.mult,
                )
                return xn

            def transpose_into(dst, xn, col):
                # bf16 transpose (fp8 PE-transpose needs stride-2 output);
                # the evac converts to dst's dtype
                for dt in range(EC):
                    pt = pstr.tile([P, P], bf16, tag="ptr")
                    nc.tensor.transpose(pt, xn[:, dt * P : (dt + 1) * P], idt)
                    nc.scalar.activation(
                        dst[:, dt, col : col + P],
                        pt,
                        mybir.ActivationFunctionType.Copy,
                    )

            def v_proj_tile(v_t, xt_v, wv, vt):
                ps = psb.tile([P, NB], f32, tag="psb")
                for ch in range(0, EC, 2):
                    nc.tensor.matmul(
                        ps,
                        xt_v[:, ch : ch + 2, vt * P : (vt + 1) * P],
                        wv[:, ch : ch + 2, :],
                        start=(ch == 0),
                        stop=(ch == EC - 2),
                        perf_mode=DR,
                    )
                nc.scalar.activation(
                    v_t[:, vt, :],
                    ps,
                    mybir.ActivationFunctionType.Identity,
                    scale=1.0 / SV,
                )

            def compute_v(xt_v, wv_d, h):
                wv = wpool.tile([P, EC, D], f8, tag="wv")
                nc.sync.dma_start(wv, wv_d[h].rearrange("(c p) d -> p c d", p=P))
                v_t = kvq.tile([P, KT, D], f8, tag="vtile")
                for vt in range(KT):
                    v_proj_tile(v_t, xt_v, wv, vt)
                return v_t

            def q2_proj_rb(q_t, wa, xt_q, rb, qb_col, h):
                for dt in range(EC):
                    ps = psb.tile([P, NB], f32, tag="psb")
                    for ch in range(0, EC, 2):
                        nc.tensor.matmul(
                            ps,
                            wa[:, ch : ch + 2, dt * P : (dt + 1) * P],
                            xt_q[:, ch : ch + 2, rb * NB : (rb + 1) * NB],
                            start=(ch == 0),
                            stop=(ch == EC - 2),
                            perf_mode=DR,
                        )
                    nc.scalar.activation(
                        q_t[:, dt, rb * NB : (rb + 1) * NB],
                        ps,
                        mybir.ActivationFunctionType.Identity,
                        bias=bia[:, qb_col + h * 4 + dt : qb_col + h * 4 + dt + 1],
                        scale=1.0 / SA,
                    )

            def attention(xt_q, xt_k, xt_v, a_d, wv_d, qb_col, causal,
                          pre_v=None, pre_q=None, rb_order=None, post_rb=None):
                """4 heads; accumulate softmax@V/8 into acc (scores via the
                host-folded [E,E] per-head matrix; keys operand = xt_k)."""
                n_rb = L // NB
                for h in range(HG):
                    if h == 0 and pre_v is not None:
                        v_t = pre_v
                    else:
                        v_t = compute_v(xt_v, wv_d, h)
                    if h == 0 and pre_q is not None:
                        q_t = pre_q
                    else:
                        wa = wpool.tile([P, EC, E], f8, tag="wq")
                        nc.sync.dma_start(
                            wa, a_d[h].rearrange("(c p) d -> p c d", p=P)
                        )
                        q_t = kvq.tile([P, EC, L], f8, tag="qtile")
                        for rb in range(n_rb):
                            q2_proj_rb(q_t, wa, xt_q, rb, qb_col, h)

                    for rb in (rb_order if rb_order is not None else range(n_rb)):
                        if causal:
                            score_kts = list(range(min(KT, (rb + 1) * (NB // P))))
                        else:
                            score_kts = list(range(KT))
                        p_t = ppool.tile([P, KT, NB], f8, tag="ptile")
                        for kt in score_kts:
                            # rows below the diagonal sub-block never read
                            # this key tile: narrow the moving operand
                            diag = causal and kt >= rb * (NB // P)
                            j = (kt - rb * (NB // P)) if diag else 0
                            w = NB - j * P
                            ps = psb.tile([P, NB], f32, tag="psb")
                            for ch in range(0, EC, 2):
                                nc.tensor.matmul(
                                    ps[:, 0:w],
                                    xt_k[:, ch : ch + 2, kt * P : (kt + 1) * P],
                                    q_t[:, ch : ch + 2,
                                        rb * NB + j * P : (rb + 1) * NB],
                                    start=(ch == 0),
                                    stop=(ch == EC - 2),
                                    perf_mode=DR,
                                )
                            nc.scalar.activation(
                                p_t[:, kt, j * P : NB],
                                ps[:, 0:w],
                                mybir.ActivationFunctionType.Exp,
                                scale=SCALE,
                            )
                            if diag:
                                # diagonal 128x128 sub-block: zero keys > row
                                nc.vector.tensor_mul(
                                    p_t[:, kt, j * P : (j + 1) * P],
                                    p_t[:, kt, j * P : (j + 1) * P],
                                    tri,
                                )
                        for i in range(NB // P):
                            rs = rb * (NB // P) + i
                            n_kt = (rs + 1) if causal else KT
                            pr = pss.tile([P, 1], f32, tag="pssum")
                            for j in range(n_kt):
                                nc.tensor.matmul(
                                    pr,
                                    p_t[:, j, i * P : (i + 1) * P],
                                    eights,
                                    start=(j == 0),
                                    stop=(j == n_kt - 1),
                                )
                            r8 = statp.tile([P, 1], f32, tag="r8")
                            nc.vector.reciprocal(r8, pr)
                            po = psb.tile([P, NB], f32, tag="psb")
                            npair = n_kt // 2
                            for j in range(npair):
                                nc.tensor.matmul(
                                    po,
                                    p_t[:, 2 * j : 2 * j + 2, i * P : (i + 1) * P],
                                    v_t[:, 2 * j : 2 * j + 2, :],
                                    start=(j == 0),
                                    stop=(j == npair - 1 and n_kt % 2 == 0),
                                    perf_mode=DR,
                                )
                            if n_kt % 2:
                                nc.tensor.matmul(
                                    po,
                                    p_t[:, n_kt - 1, i * P : (i + 1) * P],
                                    v_t[:, n_kt - 1, :],
                                    start=(npair == 0),
                                    stop=True,
                                )
                            ot = tmpp.tile([P, E], f32, tag="f32s")
                            nc.vector.tensor_scalar_mul(ot, po, r8)
                            nc.vector.tensor_add(acc[:, rs, :], acc[:, rs, :], ot)
                        if post_rb is not None:
                            post_rb(h, rb)

            def emit_ar(qtr):
                # quarter = 4 row slices = one rb of 512 rows
                for rs in range(qtr * 4, qtr * 4 + 4):
                    art = tmpp.tile([P, E], bf16, tag="bfs")
                    nc.vector.tensor_copy(art, acc[:, rs, :])
                    nc.sync.dma_start(ar_in[rs * P : (rs + 1) * P, :], art)
                nc.gpsimd.collective_compute(
                    "AllReduce",
                    mybir.AluOpType.add,
                    replica_groups=GROUPS,
                    ins=[ar_in[qtr * NB : (qtr + 1) * NB, :]],
                    outs=[ar_out[qtr * NB : (qtr + 1) * NB, :]],
                )

            def emit_rs(chunk):
                # chunk c: own slices {2c, 2c+1} + partner slices {8+2c, 8+2c+1};
                # first half of the 512-row input goes to the pair's first core
                slices = [2 * chunk, 2 * chunk + 1, 8 + 2 * chunk, 9 + 2 * chunk]
                for idx, rs in enumerate(slices):
                    pos = chunk * 512 + (idx // 2) * 256 + (idx % 2) * P
                    rcast = tmpp.tile([P, E], bf16, tag="bfs")
                    nc.vector.tensor_copy(rcast, acc[:, rs, :])
                    nc.sync.dma_start(rs_in[pos : pos + P, :], rcast)
                nc.gpsimd.collective_compute(
                    "ReduceScatter",
                    mybir.AluOpType.add,
                    replica_groups=GROUPS,
                    ins=[rs_in[chunk * 512 : (chunk + 1) * 512, :]],
                    outs=[rs_out[chunk * 256 : (chunk + 1) * 256, :]],
                )

            # ---------------- phases A-C: attention pools live here ----------------
            attn_pools = (
                tc.tile_pool(name="ffw", bufs=1),
                tc.tile_pool(name="accp", bufs=1),
                tc.tile_pool(name="wpool", bufs=2),
                tc.tile_pool(name="kvq", bufs=2),
                tc.tile_pool(name="ppool", bufs=2),
            )
            ffwp, accp, wpool, kvq, ppool = (p.__enter__() for p in attn_pools)
            ffwp, accp, wpool, kvq, ppool = [
                p for p in (ffwp, accp, wpool, kvq, ppool)
            ]
            # attention partial accumulator over ALL rows, [128, 16, 512] f32
            acc = accp.tile([P, LS, E], f32)
            nc.vector.memset(acc, 0.0)

            # prefetch cross-attention K/V and FFN weights early; the DMAs
            # drain during phase A/B
            with tc.tile_pool(name="xt2", bufs=1) as xt2p, tc.tile_pool(
                name="kvin", bufs=1
            ) as kvinp:
                ktt = kvinp.tile([P, EC, L], f8, tag="ktin")
                nc.sync.dma_start(ktt, kT_in.rearrange("(c p) t -> p c t", p=P))
                vtt = kvinp.tile([P, EC, L], f8, tag="vtin")
                nc.sync.dma_start(vtt, vT_in.rearrange("(c p) t -> p c t", p=P))
                w1t = ffwp.tile([P, EC, HID], bf16, tag="w1t")
                nc.sync.dma_start(w1t, w1_in.rearrange("(c p) d -> p c d", p=P))
                w2t = ffwp.tile([P, HC, E], bf16, tag="w2t")
                nc.sync.dma_start(w2t, w2_in.rearrange("(c p) d -> p c d", p=P))

                # ---------------- phase A: LN1 -> X^T over all rows ------------
                # head-0 V/Q2 projections interleave per slice to fill the PE
                with tc.tile_pool(name="xt1", bufs=1) as xt1p, tc.tile_pool(
                    name="qstream", bufs=3
                ) as qsp:
                    xt = xt1p.tile([P, EC, L], f8)
                    wv0 = wpool.tile([P, EC, D], f8, tag="wv")
                    nc.sync.dma_start(
                        wv0, wv_s_in[0].rearrange("(c p) d -> p c d", p=P)
                    )
                    wa0 = wpool.tile([P, EC, E], f8, tag="wq")
                    nc.sync.dma_start(
                        wa0, a_s_in[0].rearrange("(c p) d -> p c d", p=P)
                    )
                    v0 = kvq.tile([P, KT, D], f8, tag="vtile")
                    q0 = kvq.tile([P, EC, L], f8, tag="qtile")
                    for t in range(LS):
                        qt = qsp.tile([P, E], f32, tag="qs")
                        nc.sync.dma_start(qt, q_in[t * P : (t + 1) * P, :])
                        xn = ln_normalize(qt, bf16)
                        transpose_into(xt, xn, t * P)
                        v_proj_tile(v0, xt, wv0, t)
                        if t % 4 == 3:
                            q2_proj_rb(q0, wa0, xt, t // 4, CQ_S, 0)

                    # ------------- phase B: causal self-attention (4 heads) ----
                    def post_self(h, rb):
                        # AR quarter rb as soon as the last head finishes it
                        if h == HG - 1:
                            emit_ar(rb)

                    attention(xt, xt, xt, a_s_in, wv_s_in, CQ_S, True,
                              pre_v=v0, pre_q=q0, post_rb=post_self)

                # ---------------- phase C: x1, LN2, cross-attention ------------
                x2t = xt2p.tile([P, EC, L], f8)
                pre = compute_v(vtt, wv_c_in, 0)
                wac0 = wpool.tile([P, EC, E], f8, tag="wq")
                nc.sync.dma_start(wac0, a_c_in[0].rearrange("(c p) d -> p c d", p=P))
                q0c = kvq.tile([P, EC, L], f8, tag="qtile")
                # x1 is streamed one row-slice at a time; nothing reads it later
                # (acc gets x1/2, LN2 consumes it, x2 arrives via ReduceScatter)
                for rs in range(LS):
                    art = tmpp.tile([P, E], bf16, tag="bfs")
                    nc.sync.dma_start(art, ar_out[rs * P : (rs + 1) * P, :])
                    qcvt = tmpp.tile([P, E], f32, tag="f32s")
                    nc.sync.dma_start(qcvt, qcv_in[rs * P : (rs + 1) * P, :])
                    x1s = tmpp.tile([P, E], f32, tag="f32s")
                    nc.vector.tensor_add(x1s, qcvt, art)
                    xn = ln_normalize(x1s, bf16)
                    transpose_into(x2t, xn, rs * P)
                    # acc <- x1/2 so the pair's ReduceScatter sums to x2
                    nc.vector.tensor_scalar_mul(acc[:, rs, :], x1s, 0.5)
                    if rs % 4 == 3:
                        q2_proj_rb(q0c, wac0, x2t, rs // 4, CQ_C, 0)

                def post_cross(h, rb):
                    # chunks 0,1 = row slices {0-3, 8-11} = rb 0,2; with order
                    # [0,2,1,3] they complete after the last head's 2nd rb
                    if h == HG - 1 and rb == 2:
                        emit_rs(0)
                        emit_rs(1)

                attention(x2t, ktt, vtt, a_c_in, wv_c_in, CQ_C, False,
                          pre_v=pre, pre_q=q0c, rb_order=[0, 2, 1, 3],
                          post_rb=post_cross)
            emit_rs(2)
            emit_rs(3)
            for p in reversed(attn_pools[1:]):
                p.__exit__(None, None, None)

            # ---------------- phase D: FFN on own half, per 512-row chunk ------
            with tc.tile_pool(name="x2p", bufs=1) as x2p, tc.tile_pool(
                name="xt3", bufs=1
            ) as xt3p, tc.tile_pool(name="h1p", bufs=1) as h1p:
                x2b = x2p.tile([P, RS8, E], bf16)
                x2 = x2p.tile([P, RS8, E], f32)
                x3t = xt3p.tile([P, EC, HALF], bf16)
                h1t = h1p.tile([P, HC, HALF], bf16)
                HB = 256  # 2 row slices per pipelined chunk
                for chunk in range(4):
                    for rs in (2 * chunk, 2 * chunk + 1):
                        nc.sync.dma_start(
                            x2b[:, rs, :], rs_out[rs * P : (rs + 1) * P, :]
                        )
                        xn = ln_normalize(x2b[:, rs, :], bf16)
                        transpose_into(x3t, xn, rs * P)
                        nc.vector.tensor_add(x2[:, rs, :], x2b[:, rs, :], b2t)

                    for ht in range(HC):
                        ps = psb.tile([P, NB], f32, tag="psb")
                        for ch in range(EC):
                            nc.tensor.matmul(
                                ps[:, 0:HB],
                                w1t[:, ch, ht * P : (ht + 1) * P],
                                x3t[:, ch, chunk * HB : (chunk + 1) * HB],
                                start=(ch == 0),
                                stop=(ch == EC - 1),
                            )
                        nc.scalar.activation(
                            h1t[:, ht, chunk * HB : (chunk + 1) * HB],
                            ps[:, 0:HB],
                            mybir.ActivationFunctionType.Relu,
                            bias=bia[:, C1 + ht : C1 + ht + 1],
                        )

                    for rs in (2 * chunk, 2 * chunk + 1):
                        ps = psb.tile([P, NB], f32, tag="psb")
                        for ch in range(HC):
                            nc.tensor.matmul(
                                ps,
                                h1t[:, ch, rs * P : (rs + 1) * P],
                                w2t[:, ch, :],
                                start=(ch == 0),
                                stop=(ch == HC - 1),
                            )
                        ot = tmpp.tile([P, E], f32, tag="f32s")
                        nc.vector.tensor_add(ot, ps, x2[:, rs, :])
                        nc.sync.dma_start(out_d[rs * P : (rs + 1) * P, :], ot)
            attn_pools[0].__exit__(None, None, None)

    nc.compile()
    return nc


def _ensure_ntff_hook():
    try:
        from antenv.axon_hooks import get_axon_ntff_profile_hook  # noqa: F401
        return
    except ImportError:
        pass
    import antenv

    mod = types.ModuleType("antenv.axon_hooks")
    _hook = [None]
    mod.set_axon_ntff_profile_hook = lambda h: _hook.__setitem__(0, h)
    mod.get_axon_ntff_profile_hook = lambda: _hook[0]
    sys.modules["antenv.axon_hooks"] = mod
    antenv.axon_hooks = mod
    from trn_agent_boot.trn_boot import _ntff_profile_via_ctypes

    mod.set_axon_ntff_profile_hook(
        _ntff_profile_via_ctypes("/opt/axon/libaxon_pjrt.so")
    )


def _to8(x, scale=1.0):
    return np.clip(np.asarray(x, np.float32) * scale, -240.0, 240.0).astype(F8)


def kernel(**inputs):
    f = np.float32
    q = np.asarray(inputs["q"], f)
    k = np.asarray(inputs["k"], f)
    v = np.asarray(inputs["v"], f)
    Wq_s = np.asarray(inputs["Wq_s"], f)
    Wk_s = np.asarray(inputs["Wk_s"], f)
    Wv_s = np.asarray(inputs["Wv_s"], f)
    Wq_c = np.asarray(inputs["Wq_c"], f)
    Wk_c = np.asarray(inputs["Wk_c"], f)
    Wv_c = np.asarray(inputs["Wv_c"], f)
    W1 = np.asarray(inputs["W1"], f)
    b1 = np.asarray(inputs["b1"], f)
    W2 = np.asarray(inputs["W2"], f)
    b2 = np.asarray(inputs["b2"], f)
    g1 = np.asarray(inputs["g1"], f)
    be1 = np.asarray(inputs["be1"], f)
    g2 = np.asarray(inputs["g2"], f)
    be2 = np.asarray(inputs["be2"], f)
    g3 = np.asarray(inputs["g3"], f)
    be3 = np.asarray(inputs["be3"], f)

    # folded [E,E] score matrices: S = z A z_k^T (+ w . z_key per key)
    A_s = g1[None, :, None] * (Wq_s @ np.swapaxes(Wk_s, 1, 2)) * g1[None, None, :]
    w_s = np.einsum("e,hef->hf", be1, (Wq_s @ np.swapaxes(Wk_s, 1, 2))) * g1[None, :]
    A_c = g2[None, :, None] * (Wq_c @ np.swapaxes(Wk_c, 1, 2))
    w_c = np.einsum("e,hef->hf", be2, (Wq_c @ np.swapaxes(Wk_c, 1, 2)))
    WvsF = Wv_s * g1[None, :, None]
    # V-projection biases contribute mean_h(be1 @ Wv_s[h]) to every attention
    # output row (softmax rows sum to 1); pre-added to q on the host.
    cvbar = np.einsum("e,hed->d", be1, Wv_s) / H

    A_s8 = np.ascontiguousarray(_to8(A_s, SA))
    A_c8 = np.ascontiguousarray(_to8(A_c, SA))
    Wv_s8 = np.ascontiguousarray(_to8(WvsF, SV))
    Wv_c8 = np.ascontiguousarray(_to8(Wv_c, SV))
    W1F = np.ascontiguousarray((W1 * g3[:, None]).astype(BF16))
    c1 = be3 @ W1 + b1
    W2F = np.ascontiguousarray(W2.astype(BF16))

    b2rep = np.broadcast_to(b2[None, :], (P, E)).astype(f).copy()
    ident = np.eye(P, dtype=BF16)
    # tri[key_i, row_j] = 1 where key <= row within a diagonal block
    tri = np.triu(np.ones((P, P), np.float32)).astype(F8)

    in_maps = []
    for core in range(8):
        b, hg = core // 2, core % 2
        hsl = slice(hg * HG, (hg + 1) * HG)
        biases = np.zeros((P, NBIAS), f)
        for h in range(HG):
            for c in range(4):
                biases[:, CQ_S + h * 4 + c] = w_s[hsl][h, c * P : (c + 1) * P]
                biases[:, CQ_C + h * 4 + c] = w_c[hsl][h, c * P : (c + 1) * P]
        for c in range(HC):
            biases[:, C1 + c] = c1[c * P : (c + 1) * P]
        in_maps.append(
            dict(
                q_nat=np.ascontiguousarray(q[b]),
                qcv=np.ascontiguousarray(q[b] + cvbar[None, :]),
                kT=np.ascontiguousarray(_to8(k[b].T)),
                vT=np.ascontiguousarray(_to8(v[b].T)),
                trimask=tri,
                a_s=np.ascontiguousarray(A_s8[hsl]),
                wv_s=np.ascontiguousarray(Wv_s8[hsl]),
                a_c=np.ascontiguousarray(A_c8[hsl]),
                wv_c=np.ascontiguousarray(Wv_c8[hsl]),
                w1=W1F,
                w2=W2F,
                biases=biases,
                b2rep=b2rep,
                identity=ident,
            )
        )

    if "nc" not in _CACHE:
        _CACHE["nc"] = _build()
    nc = _CACHE["nc"]

    kwargs = {}
    if TRACE:
        _ensure_ntff_hook()
        import os as _os

        _os.environ["BASS_PERFETTO_PROFILE_ALL_CORES"] = "1"
        import tempfile

        kwargs = dict(trace=True, tmpdir=tempfile.mkdtemp())
    res = run_bass_kernel_spmd(nc, in_maps, core_ids=list(range(8)), **kwargs)
    _CACHE["last_res"] = res

    out = np.empty((B, L, E), f)
    for core in range(8):
        b, half = core // 2, core % 2
        out[b, half * HALF : (half + 1) * HALF] = res.results[core]["out"]
    return out
